# revision 3
# baseline (speedup 1.0000x reference)
"""Trainium2 Bass kernel for DiffFeatureMatcher softmax-matching loss.

Reference computation (fp32):
    src_idx = round/clip(src keypoints) -> [N] pixel ids
    kp      = src_flat[:, src_idx]                      [C, N]
    dist    = kp_sq[n] + tgt_sq[m] - 2*kp.T@tgt         [N, HW]
    resp    = exp(-sigma*dist); resp /= rowsum(resp)
    loss    = mean(-log(eps + resp[n, gt_idx[n]]))      scalar

Sharding: the HW (pixel) axis is split across the 8 cores -- each core
computes all N keypoints against its HW/8 pixel slice and returns the
partial softmax denominators rowsum_m exp(-sigma*dist[n, m]).  The
numerator (one pixel per keypoint) and the final log/mean are O(N*C)
and done on host in fp32, mirroring the reference op-for-op.

Device math per core, per keypoint group g (128 keypoints) and 2048-px
PSUM window:
    psum = kp_g.T @ tgt_px              (fp32r matmul, 1 cyc/row)
         + ones.T @ [bhi; blo]          (bf16 K=2 matmul; bhi+blo ~= -0.5*tgt_sq)
    exp  = Exp(2*sigma*psum + (-sigma*kp_sq[g]))   (ACT, per-partition bias AP,
                                                    scale from SBUF AP)
    accum_out += rowsum(exp)            (fused in the same ACT instruction)
The exp output tile itself is scratch; only the accumulated row sums
leave the chip ([128 partitions, 8 groups * 5 windows] -> reduced to
[128, 8] on DVE).

exp(-sigma*dist) underflows to exactly 0.0 in fp32 for this problem's
input scale (sigma*dist >= ~550), so every row-sum is 0.0 and the
reference loss is 0/0 -> NaN; the host combine reproduces that exactly.
"""

from contextlib import ExitStack

import numpy as np

# Problem constants (hardcoded per the harness contract).
C = 128
H, W = 256, 320
HW = H * W              # 81920
N = 1024
NCORES = 8
PXC = HW // NCORES      # 10240 pixels per core
NG = N // 128           # 8 keypoint groups of 128
WIN = 2048              # pixels per PSUM window (4 banks)
NWIN = PXC // WIN       # 5 windows per group
LOSS_EPS = np.float32(1e-10)

_CACHED_NC = None


def _build_nc():
    import concourse.tile as tile
    import concourse.mybir as mybir
    from concourse import bacc

    F32 = mybir.dt.float32
    F32R = mybir.dt.float32r
    BF16 = mybir.dt.bfloat16

    nc = bacc.Bacc("TRN2", target_bir_lowering=False, debug=False,
                   enable_asserts=False, num_devices=NCORES)

    tgt_d = nc.dram_tensor("tgtpx", [C, PXC], F32R, kind="ExternalInput")
    kpf_d = nc.dram_tensor("kpf", [C, N], F32R, kind="ExternalInput")
    bhl_d = nc.dram_tensor("bhl", [2, PXC], BF16, kind="ExternalInput")
    biasg_d = nc.dram_tensor("biasg", [128, NG], F32, kind="ExternalInput")
    scl_d = nc.dram_tensor("scl", [128, 1], F32, kind="ExternalInput")
    dsum_d = nc.dram_tensor("dsum", [128, NG], F32, kind="ExternalOutput")

    with tile.TileContext(nc) as tc, ExitStack() as ctx:
        const_pool = ctx.enter_context(tc.tile_pool(name="const", bufs=1))
        psum_pool = ctx.enter_context(
            tc.tile_pool(name="ps", bufs=2, space="PSUM"))
        scr_pool = ctx.enter_context(tc.tile_pool(name="scr", bufs=3))

        # Persistent SBUF residents.
        kpf_sb = const_pool.tile([C, N], F32R)
        nc.sync.dma_start(kpf_sb[:], kpf_d.ap()[:])
        bhl_sb = const_pool.tile([2, PXC], BF16)
        nc.sync.dma_start(bhl_sb[:], bhl_d.ap()[:])
        biasg_sb = const_pool.tile([128, NG], F32)
        nc.sync.dma_start(biasg_sb[:], biasg_d.ap()[:])
        scl_sb = const_pool.tile([128, 1], F32)
        nc.sync.dma_start(scl_sb[:], scl_d.ap()[:])
        ones2_sb = const_pool.tile([2, 128], BF16)
        nc.vector.memset(ones2_sb[:], 1.0)
        acc_sb = const_pool.tile([128, NG, NWIN], F32)

        # Target slice, DMA'd in window-sized tiles so compute starts early.
        tgt_tiles = []
        for w in range(NWIN):
            t = const_pool.tile([C, WIN], F32R, tag=f"tgtw{w}")
            nc.sync.dma_start(t[:], tgt_d.ap()[:, w * WIN:(w + 1) * WIN])
            tgt_tiles.append(t)

        for g in range(NG):
            kp_g = kpf_sb[:, g * 128:(g + 1) * 128]
            for w in range(NWIN):
                pst = psum_pool.tile([128, WIN], F32)
                for j in range(WIN // 512):
                    pxo = w * WIN + j * 512
                    nc.tensor.matmul(
                        pst[:, j * 512:(j + 1) * 512], kp_g,
                        tgt_tiles[w][:, j * 512:(j + 1) * 512],
                        start=True, stop=False)
                    nc.tensor.matmul(
                        pst[:, j * 512:(j + 1) * 512], ones2_sb[:],
                        bhl_sb[:, pxo:pxo + 512],
                        start=False, stop=True)
                scr = scr_pool.tile([128, WIN], F32)
                nc.scalar.activation(
                    scr[:], pst[:], mybir.ActivationFunctionType.Exp,
                    bias=biasg_sb[:, g:g + 1], scale=scl_sb[:],
                    accum_out=acc_sb[:, g, w:w + 1])

        dsum_sb = const_pool.tile([128, NG], F32)
        nc.vector.reduce_sum(dsum_sb[:], acc_sb[:], axis=mybir.AxisListType.X)
        nc.sync.dma_start(dsum_d.ap()[:], dsum_sb[:])

    nc.compile()
    return nc


def _get_nc():
    global _CACHED_NC
    if _CACHED_NC is None:
        _CACHED_NC = _build_nc()
    return _CACHED_NC


def _hw_to_1d(loc_hw):
    # round + clamp + flatten, mirroring the reference in fp32.
    h = np.clip(np.round(loc_hw[0, :, 0]), 0, H - 1)
    w = np.clip(np.round(loc_hw[0, :, 1]), 0, W - 1)
    return (np.float32(W) * h + w).astype(np.int32)


def _prepare_in_maps(src_feature_map, tgt_feature_map,
                     src_keypoint_2d_hw_locations,
                     gt_tgt_keypoint_2d_hw_locations, response_sigma):
    """Host-side shard/precompute. Returns (in_maps, host_ctx)."""
    import ml_dtypes

    src_flat = np.ascontiguousarray(
        src_feature_map.reshape(C, HW).astype(np.float32, copy=False))
    tgt_flat = np.ascontiguousarray(
        tgt_feature_map.reshape(C, HW).astype(np.float32, copy=False))
    sigma = np.float32(np.asarray(response_sigma).reshape(-1)[0])

    src_idx = _hw_to_1d(np.asarray(src_keypoint_2d_hw_locations))
    gt_idx = _hw_to_1d(np.asarray(gt_tgt_keypoint_2d_hw_locations))

    kp = np.ascontiguousarray(src_flat[:, src_idx])          # [C, N]
    kp_sq = np.einsum("cn,cn->n", kp, kp, dtype=np.float32)  # [N]
    tgt_sq = np.einsum("cm,cm->m", tgt_flat, tgt_flat, dtype=np.float32)

    # bf16 hi/lo split of -0.5*tgt_sq (exactly representable sum on device).
    brow = (np.float32(-0.5) * tgt_sq).astype(np.float32)
    bhi = brow.astype(ml_dtypes.bfloat16)
    blo = (brow - bhi.astype(np.float32)).astype(ml_dtypes.bfloat16)

    biasg = np.ascontiguousarray(
        (-sigma * kp_sq).reshape(NG, 128).T.astype(np.float32))
    scl = np.full((128, 1), np.float32(2.0) * sigma, dtype=np.float32)

    in_maps = []
    for r in range(NCORES):
        sl = slice(r * PXC, (r + 1) * PXC)
        in_maps.append({
            "tgtpx": np.ascontiguousarray(tgt_flat[:, sl]),
            "kpf": kp,
            "bhl": np.ascontiguousarray(
                np.stack([bhi[sl], blo[sl]], axis=0)),
            "biasg": biasg,
            "scl": scl,
        })

    # Numerator (one target pixel per keypoint), fp32 on host like the ref.
    tgt_g = tgt_flat[:, gt_idx]                       # [C, N]
    cross_g = np.einsum("cn,cn->n", kp, tgt_g, dtype=np.float32)
    dist_g = (kp_sq + tgt_sq[gt_idx] - np.float32(2.0) * cross_g
              ).astype(np.float32)
    numer = np.exp(-sigma * dist_g).astype(np.float32)
    return in_maps, numer


def _combine(results, numer):
    # denom[n] = sum over cores of the per-core partial row sums.
    denom = np.zeros(N, dtype=np.float32)
    for r in range(NCORES):
        dsum = results[r]["dsum"]                     # [128, NG]
        denom += dsum.T.reshape(N)
    with np.errstate(divide="ignore", invalid="ignore"):
        sampled = (numer / denom).astype(np.float32)
        loss = np.mean(-np.log(LOSS_EPS + sampled)).astype(np.float32)
    return np.asarray(loss, dtype=np.float32)


def kernel(src_feature_map, tgt_feature_map, src_keypoint_2d_hw_locations,
           gt_tgt_keypoint_2d_hw_locations, response_sigma):
    from concourse.bass_utils import run_bass_kernel_spmd

    in_maps, numer = _prepare_in_maps(
        src_feature_map, tgt_feature_map, src_keypoint_2d_hw_locations,
        gt_tgt_keypoint_2d_hw_locations, response_sigma)
    nc = _get_nc()
    res = run_bass_kernel_spmd(nc, in_maps, core_ids=list(range(NCORES)))
    return _combine(res.results, numer)


# revision 6
# speedup vs baseline: 1.7016x; 1.7016x over previous
"""Trainium2 Bass kernel for DiffFeatureMatcher softmax-matching loss.

Reference computation (fp32):
    src_idx = round/clip(src keypoints) -> [N] pixel ids
    kp      = src_flat[:, src_idx]                      [C, N]
    dist    = kp_sq[n] + tgt_sq[m] - 2*kp.T@tgt         [N, HW]
    resp    = exp(-sigma*dist); resp /= rowsum(resp)
    loss    = mean(-log(eps + resp[n, gt_idx[n]]))      scalar

Sharding: the HW (pixel) axis is split across the 8 cores -- each core
computes all N keypoints against its HW/8 pixel slice and returns the
partial softmax denominators rowsum_m exp(-sigma*dist[n, m]).  The
numerator (one pixel per keypoint) and the final log/mean are O(N*C)
and done on host in fp32, mirroring the reference op-for-op.

Device math per core, per keypoint group g (128 keypoints) and 2048-px
PSUM window:
    psum = kp_g.T @ tgt_px              (fp32r matmul, 1 cyc/row)
         + ones.T @ [bhi; blo]          (bf16 K=2 matmul; bhi+blo ~= -0.5*tgt_sq)
    exp  = Exp(2*sigma*psum + (-sigma*kp_sq[g]))   (ACT, per-partition bias AP,
                                                    scale from SBUF AP)
    accum_out += rowsum(exp)            (fused in the same ACT instruction)
The exp output tile itself is scratch; only the accumulated row sums
leave the chip ([128 partitions, 8 groups * 5 windows] -> reduced to
[128, 8] on DVE).

exp(-sigma*dist) underflows to exactly 0.0 in fp32 for this problem's
input scale (sigma*dist >= ~550), so every row-sum is 0.0 and the
reference loss is 0/0 -> NaN; the host combine reproduces that exactly.
"""

from contextlib import ExitStack

import numpy as np

# Problem constants (hardcoded per the harness contract).
C = 128
H, W = 256, 320
HW = H * W              # 81920
N = 1024
NCORES = 8
PXC = HW // NCORES      # 10240 pixels per core
NG = N // 128           # 8 keypoint groups of 128
WIN = 2048              # pixels per PSUM window (4 banks)
NWIN = PXC // WIN       # 5 windows per group
LOSS_EPS = np.float32(1e-10)

_CACHED_NC = None


def _build_nc():
    import concourse.tile as tile
    import concourse.mybir as mybir
    from concourse import bacc

    F32 = mybir.dt.float32
    F32R = mybir.dt.float32r
    BF16 = mybir.dt.bfloat16

    nc = bacc.Bacc("TRN2", target_bir_lowering=False, debug=False,
                   enable_asserts=False, num_devices=NCORES)

    tgt_d = nc.dram_tensor("tgtpx", [C, PXC], F32R, kind="ExternalInput")
    kpf_d = nc.dram_tensor("kpf", [C, N], F32R, kind="ExternalInput")
    bcast_d = nc.dram_tensor("bcast", [128, PXC], F32, kind="ExternalInput")
    biasg_d = nc.dram_tensor("biasg", [128, NG], F32, kind="ExternalInput")
    scl_d = nc.dram_tensor("scl", [128, 1], F32, kind="ExternalInput")
    dsum_d = nc.dram_tensor("dsum", [128, NG], F32, kind="ExternalOutput")

    with tile.TileContext(nc) as tc, ExitStack() as ctx:
        const_pool = ctx.enter_context(tc.tile_pool(name="const", bufs=1))
        psum_pool = ctx.enter_context(
            tc.tile_pool(name="ps", bufs=2, space="PSUM"))
        expin_pool = ctx.enter_context(tc.tile_pool(name="expin", bufs=2))

        # Persistent SBUF residents.
        kpf_sb = const_pool.tile([C, N], F32R)
        nc.sync.dma_start(kpf_sb[:], kpf_d.ap()[:])
        biasg_sb = const_pool.tile([128, NG], F32)
        nc.sync.dma_start(biasg_sb[:], biasg_d.ap()[:])
        scl_sb = const_pool.tile([128, 1], F32)
        nc.sync.dma_start(scl_sb[:], scl_d.ap()[:])
        acc_sb = const_pool.tile([128, NG], F32)

        # Per-window tiles of the target slice and of the broadcast
        # -0.5*tgt_sq bias, so compute starts after the first window lands.
        tgt_tiles, bc_tiles = [], []
        for w in range(NWIN):
            t = const_pool.tile([C, WIN], F32R, tag=f"tgtw{w}")
            nc.sync.dma_start(t[:], tgt_d.ap()[:, w * WIN:(w + 1) * WIN])
            tgt_tiles.append(t)
            b = const_pool.tile([128, WIN], F32, tag=f"bcw{w}")
            nc.sync.dma_start(b[:], bcast_d.ap()[:, w * WIN:(w + 1) * WIN])
            bc_tiles.append(b)

        for g in range(NG):
            kp_g = kpf_sb[:, g * 128:(g + 1) * 128]
            expin = expin_pool.tile([128, PXC], F32)
            for w in range(NWIN):
                pst = psum_pool.tile([128, WIN], F32)
                for j in range(WIN // 512):
                    nc.tensor.matmul(
                        pst[:, j * 512:(j + 1) * 512], kp_g,
                        tgt_tiles[w][:, j * 512:(j + 1) * 512],
                        start=True, stop=True)
                # psum -> sbuf move fused with the -0.5*tgt_sq bias add
                nc.vector.tensor_add(
                    expin[:, w * WIN:(w + 1) * WIN], pst[:], bc_tiles[w][:])
            # One mega Exp per group, in-place, with fused row-sum.
            nc.scalar.activation(
                expin[:], expin[:], mybir.ActivationFunctionType.Exp,
                bias=biasg_sb[:, g:g + 1], scale=scl_sb[:],
                accum_out=acc_sb[:, g:g + 1])

        nc.sync.dma_start(dsum_d.ap()[:], acc_sb[:])

    nc.compile()
    return nc


def _get_nc():
    global _CACHED_NC
    if _CACHED_NC is None:
        _CACHED_NC = _build_nc()
    return _CACHED_NC


def _hw_to_1d(loc_hw):
    # round + clamp + flatten, mirroring the reference in fp32.
    h = np.clip(np.round(loc_hw[0, :, 0]), 0, H - 1)
    w = np.clip(np.round(loc_hw[0, :, 1]), 0, W - 1)
    return (np.float32(W) * h + w).astype(np.int32)


def _prepare_in_maps(src_feature_map, tgt_feature_map,
                     src_keypoint_2d_hw_locations,
                     gt_tgt_keypoint_2d_hw_locations, response_sigma):
    """Host-side shard/precompute. Returns (in_maps, host_ctx)."""
    import ml_dtypes

    src_flat = np.ascontiguousarray(
        src_feature_map.reshape(C, HW).astype(np.float32, copy=False))
    tgt_flat = np.ascontiguousarray(
        tgt_feature_map.reshape(C, HW).astype(np.float32, copy=False))
    sigma = np.float32(np.asarray(response_sigma).reshape(-1)[0])

    src_idx = _hw_to_1d(np.asarray(src_keypoint_2d_hw_locations))
    gt_idx = _hw_to_1d(np.asarray(gt_tgt_keypoint_2d_hw_locations))

    kp = np.ascontiguousarray(src_flat[:, src_idx])          # [C, N]
    kp_sq = np.einsum("cn,cn->n", kp, kp, dtype=np.float32)  # [N]
    tgt_sq = np.einsum("cm,cm->m", tgt_flat, tgt_flat, dtype=np.float32)

    brow = (np.float32(-0.5) * tgt_sq).astype(np.float32)

    biasg = np.ascontiguousarray(
        (-sigma * kp_sq).reshape(NG, 128).T.astype(np.float32))
    scl = np.full((128, 1), np.float32(2.0) * sigma, dtype=np.float32)

    in_maps = []
    for r in range(NCORES):
        sl = slice(r * PXC, (r + 1) * PXC)
        in_maps.append({
            "tgtpx": np.ascontiguousarray(tgt_flat[:, sl]),
            "kpf": kp,
            "bcast": np.ascontiguousarray(
                np.broadcast_to(brow[sl][None, :], (128, PXC))),
            "biasg": biasg,
            "scl": scl,
        })

    # Numerator (one target pixel per keypoint), fp32 on host like the ref.
    tgt_g = tgt_flat[:, gt_idx]                       # [C, N]
    cross_g = np.einsum("cn,cn->n", kp, tgt_g, dtype=np.float32)
    dist_g = (kp_sq + tgt_sq[gt_idx] - np.float32(2.0) * cross_g
              ).astype(np.float32)
    numer = np.exp(-sigma * dist_g).astype(np.float32)
    return in_maps, numer


def _combine(results, numer):
    # denom[n] = sum over cores of the per-core partial row sums.
    denom = np.zeros(N, dtype=np.float32)
    for r in range(NCORES):
        dsum = results[r]["dsum"]                     # [128, NG]
        denom += dsum.T.reshape(N)
    with np.errstate(divide="ignore", invalid="ignore"):
        sampled = (numer / denom).astype(np.float32)
        loss = np.mean(-np.log(LOSS_EPS + sampled)).astype(np.float32)
    return np.asarray(loss, dtype=np.float32)


def kernel(src_feature_map, tgt_feature_map, src_keypoint_2d_hw_locations,
           gt_tgt_keypoint_2d_hw_locations, response_sigma):
    from concourse.bass_utils import run_bass_kernel_spmd

    in_maps, numer = _prepare_in_maps(
        src_feature_map, tgt_feature_map, src_keypoint_2d_hw_locations,
        gt_tgt_keypoint_2d_hw_locations, response_sigma)
    nc = _get_nc()
    res = run_bass_kernel_spmd(nc, in_maps, core_ids=list(range(NCORES)))
    return _combine(res.results, numer)


# revision 10
# speedup vs baseline: 1.8590x; 1.0925x over previous
"""Trainium2 Bass kernel for DiffFeatureMatcher softmax-matching loss.

Reference computation (fp32):
    src_idx = round/clip(src keypoints) -> [N] pixel ids
    kp      = src_flat[:, src_idx]                      [C, N]
    dist    = kp_sq[n] + tgt_sq[m] - 2*kp.T@tgt         [N, HW]
    resp    = exp(-sigma*dist); resp /= rowsum(resp)
    loss    = mean(-log(eps + resp[n, gt_idx[n]]))      scalar

Sharding: the HW (pixel) axis is split across the 8 cores -- each core
computes all N keypoints against its HW/8 pixel slice and returns the
partial softmax denominators rowsum_m exp(-sigma*dist[n, m]).  The
numerator (one pixel per keypoint) and the final log/mean are O(N*C)
and done on host in fp32, mirroring the reference op-for-op.

Device math per core, per keypoint group g (128 keypoints) and 2048-px
PSUM window:
    psum = kp_g.T @ tgt_px              (fp32r matmul, 1 cyc/row)
         + ones.T @ [bhi; blo]          (bf16 K=2 matmul; bhi+blo ~= -0.5*tgt_sq)
    exp  = Exp(2*sigma*psum + (-sigma*kp_sq[g]))   (ACT, per-partition bias AP,
                                                    scale from SBUF AP)
    accum_out += rowsum(exp)            (fused in the same ACT instruction)
The exp output tile itself is scratch; only the accumulated row sums
leave the chip ([128 partitions, 8 groups * 5 windows] -> reduced to
[128, 8] on DVE).

exp(-sigma*dist) underflows to exactly 0.0 in fp32 for this problem's
input scale (sigma*dist >= ~550), so every row-sum is 0.0 and the
reference loss is 0/0 -> NaN; the host combine reproduces that exactly.
"""

from contextlib import ExitStack

import numpy as np

# Problem constants (hardcoded per the harness contract).
C = 128
H, W = 256, 320
HW = H * W              # 81920
N = 1024
NCORES = 8
PXC = HW // NCORES      # 10240 pixels per core
NG = N // 128           # 8 keypoint groups of 128
WIN = 2048              # pixels per PSUM window (4 banks)
NWIN = PXC // WIN       # 5 windows per group
LOSS_EPS = np.float32(1e-10)

_CACHED_NC = None
MODE = "W"          # "W": bias on PE (batched bf16 K=128 mm2), ACT reads PSUM
                    # "X": bias+move on DVE, ACT reads SBUF mega-tiles


def _build_nc():
    import concourse.tile as tile
    import concourse.mybir as mybir
    from concourse import bacc

    F32 = mybir.dt.float32
    F32R = mybir.dt.float32r
    BF16 = mybir.dt.bfloat16

    nc = bacc.Bacc("TRN2", target_bir_lowering=False, debug=False,
                   enable_asserts=False, num_devices=NCORES)

    tgt_d = nc.dram_tensor("tgtpx", [C, PXC], F32R, kind="ExternalInput")
    kpf_d = nc.dram_tensor("kpf", [C, N], F32R, kind="ExternalInput")
    bcast_d = nc.dram_tensor("bcast", [128, PXC],
                             F32 if MODE == "X" else BF16,
                             kind="ExternalInput")
    if MODE == "W":
        onbig_d = nc.dram_tensor("onbig", [128, 128], BF16,
                                 kind="ExternalInput")
    biasg_d = nc.dram_tensor("biasg", [128, NG], F32, kind="ExternalInput")
    scl_d = nc.dram_tensor("scl", [128, 1], F32, kind="ExternalInput")
    dsum_d = nc.dram_tensor("dsum", [128, NG], F32, kind="ExternalOutput")

    with tile.TileContext(nc) as tc, ExitStack() as ctx:
        const_pool = ctx.enter_context(tc.tile_pool(name="const", bufs=1))
        psum_pool = ctx.enter_context(
            tc.tile_pool(name="ps", bufs=2, space="PSUM"))
        scr_bufs = 2 if MODE == "X" else 3
        expin_pool = ctx.enter_context(
            tc.tile_pool(name="expin", bufs=scr_bufs))

        # Persistent SBUF residents.
        kpf_sb = const_pool.tile([C, N], F32R)
        nc.sync.dma_start(kpf_sb[:], kpf_d.ap()[:])
        biasg_sb = const_pool.tile([128, NG], F32)
        nc.sync.dma_start(biasg_sb[:], biasg_d.ap()[:])
        scl_sb = const_pool.tile([128, 1], F32)
        nc.sync.dma_start(scl_sb[:], scl_d.ap()[:])
        if MODE == "W":
            onbig_sb = const_pool.tile([128, 128], BF16)
            nc.sync.dma_start(onbig_sb[:], onbig_d.ap()[:])

        # Per-window tiles of the target slice and of the broadcast
        # -0.5*tgt_sq bias, so compute starts after the first window lands.
        tgt_tiles, bc_tiles = [], []
        for w in range(NWIN):
            t = const_pool.tile([C, WIN], F32R, tag=f"tgtw{w}")
            if w == 0:
                # First window in 512-col chunks so matmuls start sooner.
                for j in range(WIN // 512):
                    nc.sync.dma_start(t[:, j * 512:(j + 1) * 512],
                                      tgt_d.ap()[:, j * 512:(j + 1) * 512])
            else:
                nc.sync.dma_start(t[:], tgt_d.ap()[:, w * WIN:(w + 1) * WIN])
            tgt_tiles.append(t)
            b = const_pool.tile([128, WIN], F32 if MODE == "X" else BF16,
                                tag=f"bcw{w}")
            nc.sync.dma_start(b[:], bcast_d.ap()[:, w * WIN:(w + 1) * WIN])
            bc_tiles.append(b)

        if MODE == "X":
            acc_sb = const_pool.tile([128, NG], F32)
            for g in range(NG):
                kp_g = kpf_sb[:, g * 128:(g + 1) * 128]
                expin = expin_pool.tile([128, PXC], F32)
                for w in range(NWIN):
                    pst = psum_pool.tile([128, WIN], F32)
                    for j in range(WIN // 512):
                        nc.tensor.matmul(
                            pst[:, j * 512:(j + 1) * 512], kp_g,
                            tgt_tiles[w][:, j * 512:(j + 1) * 512],
                            start=True, stop=True)
                    # psum -> sbuf move fused with the -0.5*tgt_sq bias add
                    nc.vector.tensor_add(
                        expin[:, w * WIN:(w + 1) * WIN], pst[:],
                        bc_tiles[w][:])
                # One mega Exp per group, in-place, with fused row-sum.
                nc.scalar.activation(
                    expin[:], expin[:], mybir.ActivationFunctionType.Exp,
                    bias=biasg_sb[:, g:g + 1], scale=scl_sb[:],
                    accum_out=acc_sb[:, g:g + 1])
        else:
            # MODE W: per 2-window super-tile, run 8x fp32r cross matmuls,
            # then 8x bf16 K=128 bias matmuls (rhs rows 0-1 = hi/lo split of
            # -0.5*tgt_sq, rows 2-127 zero), so PE switches weights only
            # twice per 4096 px.  ACT exps straight from PSUM.
            acc_sb = const_pool.tile([128, NG, NWIN], F32)
            for g in range(NG):
                kp_g = kpf_sb[:, g * 128:(g + 1) * 128]
                for w in range(NWIN):
                    pst = psum_pool.tile([128, WIN], F32, tag="pst")
                    for j in range(WIN // 512):
                        nc.tensor.matmul(
                            pst[:, j * 512:(j + 1) * 512], kp_g,
                            tgt_tiles[w][:, j * 512:(j + 1) * 512],
                            start=True, stop=False)
                    for j in range(WIN // 512):
                        nc.tensor.matmul(
                            pst[:, j * 512:(j + 1) * 512], onbig_sb[:],
                            bc_tiles[w][:, j * 512:(j + 1) * 512],
                            start=False, stop=True)
                    scr = expin_pool.tile([128, WIN], F32, tag="scr")
                    nc.scalar.activation(
                        scr[:], pst[:], mybir.ActivationFunctionType.Exp,
                        bias=biasg_sb[:, g:g + 1], scale=scl_sb[:],
                        accum_out=acc_sb[:, g, w:w + 1])
            dsum_sb = const_pool.tile([128, NG], F32)
            nc.vector.reduce_sum(dsum_sb[:], acc_sb[:],
                                 axis=mybir.AxisListType.X)
            acc_sb = dsum_sb

        nc.sync.dma_start(dsum_d.ap()[:], acc_sb[:])

    nc.compile()
    return nc


def _get_nc():
    global _CACHED_NC
    if _CACHED_NC is None:
        _CACHED_NC = _build_nc()
    return _CACHED_NC


def _hw_to_1d(loc_hw):
    # round + clamp + flatten, mirroring the reference in fp32.
    h = np.clip(np.round(loc_hw[0, :, 0]), 0, H - 1)
    w = np.clip(np.round(loc_hw[0, :, 1]), 0, W - 1)
    return (np.float32(W) * h + w).astype(np.int32)


def _pad_rows(bhi, blo):
    import ml_dtypes
    out = np.zeros((128, bhi.shape[0]), dtype=ml_dtypes.bfloat16)
    out[0] = bhi
    out[1] = blo
    return out


def _prepare_in_maps(src_feature_map, tgt_feature_map,
                     src_keypoint_2d_hw_locations,
                     gt_tgt_keypoint_2d_hw_locations, response_sigma):
    """Host-side shard/precompute. Returns (in_maps, host_ctx)."""
    import ml_dtypes

    src_flat = np.ascontiguousarray(
        src_feature_map.reshape(C, HW).astype(np.float32, copy=False))
    tgt_flat = np.ascontiguousarray(
        tgt_feature_map.reshape(C, HW).astype(np.float32, copy=False))
    sigma = np.float32(np.asarray(response_sigma).reshape(-1)[0])

    src_idx = _hw_to_1d(np.asarray(src_keypoint_2d_hw_locations))
    gt_idx = _hw_to_1d(np.asarray(gt_tgt_keypoint_2d_hw_locations))

    kp = np.ascontiguousarray(src_flat[:, src_idx])          # [C, N]
    kp_sq = np.einsum("cn,cn->n", kp, kp, dtype=np.float32)  # [N]
    tgt_sq = np.einsum("cm,cm->m", tgt_flat, tgt_flat, dtype=np.float32)

    brow = (np.float32(-0.5) * tgt_sq).astype(np.float32)
    if MODE == "W":
        # hi/lo bf16 split rows (sum restores brow to ~2^-17 rel)
        bhi = brow.astype(ml_dtypes.bfloat16)
        blo = (brow - bhi.astype(np.float32)).astype(ml_dtypes.bfloat16)

    biasg = np.ascontiguousarray(
        (-sigma * kp_sq).reshape(NG, 128).T.astype(np.float32))
    scl = np.full((128, 1), np.float32(2.0) * sigma, dtype=np.float32)

    in_maps = []
    for r in range(NCORES):
        sl = slice(r * PXC, (r + 1) * PXC)
        in_maps.append({
            "tgtpx": np.ascontiguousarray(tgt_flat[:, sl]),
            "kpf": kp,
            "bcast": np.ascontiguousarray(
                np.broadcast_to(brow[sl][None, :], (128, PXC)))
            if MODE == "X" else _pad_rows(bhi[sl], blo[sl]),
            "biasg": biasg,
            "scl": scl,
        })
        if MODE == "W":
            in_maps[-1]["onbig"] = np.ones((128, 128), np.float32).astype(
                ml_dtypes.bfloat16)

    # Numerator (one target pixel per keypoint), fp32 on host like the ref.
    tgt_g = tgt_flat[:, gt_idx]                       # [C, N]
    cross_g = np.einsum("cn,cn->n", kp, tgt_g, dtype=np.float32)
    dist_g = (kp_sq + tgt_sq[gt_idx] - np.float32(2.0) * cross_g
              ).astype(np.float32)
    numer = np.exp(-sigma * dist_g).astype(np.float32)
    return in_maps, numer


def _combine(results, numer):
    # denom[n] = sum over cores of the per-core partial row sums.
    denom = np.zeros(N, dtype=np.float32)
    for r in range(NCORES):
        dsum = results[r]["dsum"]                     # [128, NG]
        denom += dsum.T.reshape(N)
    with np.errstate(divide="ignore", invalid="ignore"):
        sampled = (numer / denom).astype(np.float32)
        loss = np.mean(-np.log(LOSS_EPS + sampled)).astype(np.float32)
    return np.asarray(loss, dtype=np.float32)


def kernel(src_feature_map, tgt_feature_map, src_keypoint_2d_hw_locations,
           gt_tgt_keypoint_2d_hw_locations, response_sigma):
    from concourse.bass_utils import run_bass_kernel_spmd

    in_maps, numer = _prepare_in_maps(
        src_feature_map, tgt_feature_map, src_keypoint_2d_hw_locations,
        gt_tgt_keypoint_2d_hw_locations, response_sigma)
    nc = _get_nc()
    res = run_bass_kernel_spmd(nc, in_maps, core_ids=list(range(NCORES)))
    return _combine(res.results, numer)
